# revision 7
# baseline (speedup 1.0000x reference)
"""Trainium2 Bass kernel: per-pixel channel shuffle + 3x3 conv (stride 1, pad 1).

Problem: x [32,256,56,56] f32, w [256,256,3,3] f32 (OIHW), perm [3136,256] i32;
out[b,:,h,w] = conv3x3(xs)[b,:,h,w] where xs[b,:,l] = x[b, perm[l,:], l].

Strategy (8 NeuronCores, data-parallel over batch, 4 batches/core):
  host: pre-transpose x to [B, HW, C] bf16 so the per-pixel channel gather
        reads pixel-major tiles straight from DMA (no on-device forward
        transpose); build inverse-perm int16 tables batched 4 pixel-tiles
        per GPSIMD local_scatter call; pre-transpose w into 36 [128,128]
        bf16 lhsT tiles.
  device, per batch (pipelined across engines):
    step s (7 per batch): GPSIMD local_scatter shuffles 448 pixels
    (4 tiles x 112) x 256 ch in one call; PE transposes the result back to
    [c, l] (8 x [112,128] via identity matmul into PSUM); DVE copies the
    8 image rows into a zero-padded 58x58 flat image. Conv groups (18
    accumulating matmuls, N=464) for group g=s-2 are issued first in each
    step so the PE streams matmuls while GPSIMD scatters ahead.
"""

import os
import sys
import types
import numpy as np

_STATE = {}
LAST_RESULT = None

B, C, H, W = 32, 256, 56, 56
HW = H * W
PADW = 58
XS_LEN = 3376
TL = 112
NT = 28
NSC = 7          # scatter steps per batch (4 pixel-tiles each)
K4 = 4           # pixel-tiles per local_scatter call
N_GROUP = 464
N_GROUPS = 7
N_CORES = 8
B_LOC = B // N_CORES


def _install_ntff_shim():
    # antenv.axon_hooks is absent in some images; provide it so trace=True
    # (BASS_TRACE=1) can capture NTFF profiles instead of crashing.
    name = "antenv.axon_hooks"
    if name in sys.modules:
        return
    try:
        import antenv  # noqa: F401

        m = types.ModuleType(name)
        m._hook = None
        m.set_axon_ntff_profile_hook = lambda h: setattr(m, "_hook", h)
        m.get_axon_ntff_profile_hook = lambda: m._hook
        sys.modules[name] = m
        setattr(sys.modules["antenv"], "axon_hooks", m)
        from trn_agent_boot.trn_boot import _ntff_profile_via_ctypes

        hook = _ntff_profile_via_ctypes("/opt/axon/libaxon_pjrt.so")
        if hook is not None:
            m.set_axon_ntff_profile_hook(hook)
    except Exception:
        pass


def _build_kernel():
    import concourse.bass as bass
    import concourse.mybir as mybir
    from concourse import bacc, tile
    from concourse.masks import make_identity
    from contextlib import ExitStack

    F32 = mybir.dt.float32
    BF16 = mybir.dt.bfloat16
    I16 = mybir.dt.int16

    nc = bacc.Bacc("TRN2", target_bir_lowering=False, debug=False, num_devices=N_CORES)

    xb = nc.dram_tensor("xb", [B_LOC, HW, C], BF16, kind="ExternalInput")
    wt = nc.dram_tensor("wt", [36, 128, 128], BF16, kind="ExternalInput")
    idxt = nc.dram_tensor("idxt", [128, NSC * K4 * 256], I16, kind="ExternalInput")
    out = nc.dram_tensor("out", [B_LOC, C, HW], F32, kind="ExternalOutput")

    with tile.TileContext(nc) as tc, ExitStack() as ctx:
        const = ctx.enter_context(tc.tile_pool(name="const", bufs=1))
        wsb = const.tile([128, 36 * 128], BF16)
        idxsb = const.tile([128, NSC * 1024], I16)
        ident = const.tile([128, 128], BF16)
        make_identity(nc, ident[:, :])
        # first scatter only needs idx chunk 0 + xin chunk 0: split the
        # const DMAs into chunks and spread them over both HWDGE queues so
        # the pipeline starts ~1.5us in instead of waiting for 4.6MB.
        nc.sync.dma_start(out=idxsb[:, 0:1024], in_=idxt[:, 0:1024])
        for s in range(1, NSC):
            nc.scalar.dma_start(
                out=idxsb[:, s * 1024 : (s + 1) * 1024],
                in_=idxt[:, s * 1024 : (s + 1) * 1024],
            )
        nc.scalar.dma_start(
            out=wsb[:, :],
            in_=bass.AP(wt, 0, [[128, 128], [128 * 128, 36], [1, 128]]),
        )

        xin_pool = ctx.enter_context(tc.tile_pool(name="xin", bufs=2))
        xs_pool = ctx.enter_context(tc.tile_pool(name="xs", bufs=2))
        sout_pool = ctx.enter_context(tc.tile_pool(name="sout", bufs=4))
        outst_pool = ctx.enter_context(tc.tile_pool(name="outst", bufs=4))
        tps_pool = ctx.enter_context(tc.tile_pool(name="tps", bufs=2, space="PSUM"))
        mpsum_pool = ctx.enter_context(tc.tile_pool(name="mpsum", bufs=4, space="PSUM"))

        xin_t = [None] * B_LOC

        def issue_xin_dma(b):
            xin_t[b] = xin_pool.tile([128, NT * 256], BF16, name="xin", tag="xin")
            for s in range(NSC):
                nc.sync.dma_start(
                    out=xin_t[b][0:TL, s * 1024 : (s + 1) * 1024],
                    in_=bass.AP(
                        xb,
                        b * HW * C + s * K4 * TL * 256,
                        [[256, TL], [TL * 256, K4], [1, 256]],
                    ),
                )

        def conv_group(b, xs, oct, g):
            mp = mpsum_pool.tile([128, N_GROUP], F32)
            for i in range(18):
                ct, tap = divmod(i, 9)
                dh, dw = divmod(tap, 3)
                delta = (dh - 1) * PADW + (dw - 1)
                q0 = 59 + g * N_GROUP + delta
                widx = (ct * 9 + tap) * 2 + oct
                nc.tensor.matmul(
                    mp[:, :],
                    lhsT=wsb[:, widx * 128 : (widx + 1) * 128],
                    rhs=xs[:, ct * XS_LEN + q0 : ct * XS_LEN + q0 + N_GROUP],
                    start=(i == 0),
                    stop=(i == 17),
                )
            ost = outst_pool.tile([128, N_GROUP], F32)
            if oct == 0:
                nc.scalar.copy(ost[:, :], mp[:, :])
            else:
                nc.vector.tensor_copy(ost[:, :], mp[:, :])
            nc.sync.dma_start(
                out=out[b, oct * 128 : (oct + 1) * 128, g * 448 : (g + 1) * 448],
                in_=ost[:, :].rearrange("p (r x) -> p r x", r=8)[:, :, 0:56],
            )

        issue_xin_dma(0)
        for b in range(B_LOC):
            if b + 1 < B_LOC:
                issue_xin_dma(b + 1)
            xin = xin_t[b]

            xs = xs_pool.tile([128, 2 * XS_LEN], BF16, name="xs", tag="xs")
            for ct in range(2):
                base = ct * XS_LEN
                nc.vector.memset(xs[:, base : base + PADW], 0.0)
                nc.vector.memset(xs[:, base + 57 * PADW : base + XS_LEN], 0.0)
                nc.vector.memset(
                    xs[:, base + PADW : base + PADW + 56 * PADW].rearrange(
                        "p (r x) -> p r x", r=56
                    )[:, :, 0:1],
                    0.0,
                )
                nc.vector.memset(
                    xs[:, base + PADW + 57 : base + PADW + 57 + 56 * PADW].rearrange(
                        "p (r x) -> p r x", r=56
                    )[:, :, 0:1],
                    0.0,
                )

            for s in range(NSC):
                # conv groups first in program order: the PE streams these
                # 36 matmuls while GPSIMD runs scatter s concurrently.
                if s >= 2:
                    conv_group(b, xs, 0, s - 2)
                    conv_group(b, xs, 1, s - 2)
                sout = sout_pool.tile([128, K4 * 256], BF16, name="sout", tag="sout")
                nc.gpsimd.local_scatter(
                    out_ap=sout[0:TL, :],
                    data_ap=xin[0:TL, s * 1024 : (s + 1) * 1024],
                    idxs_ap=idxsb[0:TL, s * 1024 : (s + 1) * 1024],
                    channels=TL,
                    num_elems=K4 * 256,
                    num_idxs=K4 * 256,
                )
                ps2 = tps_pool.tile([128, 2 * 448], BF16, name="ps2", tag="ps2")
                for ct in range(2):
                    for k in range(K4):
                        nc.tensor.transpose(
                            ps2[:, ct * 448 + k * TL : ct * 448 + (k + 1) * TL],
                            sout[0:TL, k * 256 + ct * 128 : k * 256 + ct * 128 + 128],
                            ident[0:TL, 0:TL],
                        )
                q = 59 + 8 * s * PADW
                for ct in range(2):
                    nc.vector.tensor_copy(
                        xs[:, ct * XS_LEN + q : ct * XS_LEN + q + 8 * PADW].rearrange(
                            "p (r x) -> p r x", r=8
                        )[:, :, 0:56],
                        ps2[:, ct * 448 : (ct + 1) * 448].rearrange(
                            "p (r x) -> p r x", r=8
                        ),
                    )

            for g in (5, 6):
                conv_group(b, xs, 0, g)
                conv_group(b, xs, 1, g)

    nc.compile()
    return nc


def _host_prep(x, w, perm):
    import ml_dtypes

    # [B, C, H, W] -> [B, HW, C] bf16 (pixel-major so scatter tiles DMA
    # straight into [pixel, channel] layout)
    xf = np.ascontiguousarray(
        x.reshape(B, C, HW).transpose(0, 2, 1)
    ).astype(ml_dtypes.bfloat16)

    wtl = np.empty((36, 128, 128), dtype=ml_dtypes.bfloat16)
    wf = np.asarray(w, dtype=np.float32)
    for ct in range(2):
        for tap in range(9):
            kh, kw = divmod(tap, 3)
            for oct in range(2):
                i = (ct * 9 + tap) * 2 + oct
                wtl[i] = wf[
                    oct * 128 : (oct + 1) * 128, ct * 128 : (ct + 1) * 128, kh, kw
                ].T.astype(ml_dtypes.bfloat16)

    iperm = np.empty((HW, C), dtype=np.int16)
    np.put_along_axis(
        iperm, perm.astype(np.int64), np.arange(C, dtype=np.int16)[None, :], axis=1
    )
    idxt = np.zeros((128, NSC * 1024), dtype=np.int16)
    for s in range(NSC):
        for k in range(K4):
            t = K4 * s + k
            idxt[0:TL, s * 1024 + k * 256 : s * 1024 + (k + 1) * 256] = (
                iperm[t * TL : (t + 1) * TL, :] + k * 256
            )

    in_maps = []
    for cidx in range(N_CORES):
        in_maps.append(
            {
                "xb": np.ascontiguousarray(xf[cidx * B_LOC : (cidx + 1) * B_LOC]),
                "wt": wtl,
                "idxt": idxt,
            }
        )
    return in_maps


def kernel(x, w, perm):
    global LAST_RESULT
    _install_ntff_shim()
    from concourse.bass_utils import run_bass_kernel_spmd

    x = np.asarray(x, dtype=np.float32)
    w = np.asarray(w, dtype=np.float32)
    perm = np.asarray(perm)

    if "nc" not in _STATE:
        _STATE["nc"] = _build_kernel()
    nc = _STATE["nc"]

    in_maps = _host_prep(x, w, perm)
    res = run_bass_kernel_spmd(nc, in_maps, core_ids=list(range(N_CORES)))
    LAST_RESULT = res
    out = np.concatenate(
        [r["out"].reshape(B_LOC, C, H, W) for r in res.results], axis=0
    )
    return out.astype(np.float32)


# revision 10
# speedup vs baseline: 1.1229x; 1.1229x over previous
"""Trainium2 Bass kernel: per-pixel channel shuffle + 3x3 conv (stride 1, pad 1).

Problem: x [32,256,56,56] f32, w [256,256,3,3] f32 (OIHW), perm [3136,256] i32;
out[b,:,h,w] = conv3x3(xs)[b,:,h,w] where xs[b,:,l] = x[b, perm[l,:], l].

Strategy (8 NeuronCores, data-parallel over batch, 4 batches/core):
  host: pre-transpose x to [B, HW, C] bf16 so the per-pixel channel gather
        reads pixel-major tiles straight from DMA (no on-device forward
        transpose); build inverse-perm int16 tables batched 4 pixel-tiles
        per GPSIMD local_scatter call; pre-transpose w into 36 [128,128]
        bf16 lhsT tiles.
  device, per batch (pipelined across engines):
    step s (7 per batch): GPSIMD local_scatter shuffles 448 pixels
    (4 tiles x 112) x 256 ch in one call; PE transposes the result back to
    [c, l] (8 x [112,128] via identity matmul into PSUM); DVE copies the
    8 image rows into a zero-padded 58x58 flat image. Conv groups (18
    accumulating matmuls, N=464) for group g=s-2 are issued first in each
    step so the PE streams matmuls while GPSIMD scatters ahead.
"""

import os
import sys
import types
import numpy as np

_STATE = {}
LAST_RESULT = None

B, C, H, W = 32, 256, 56, 56
HW = H * W
PADW = 58
XS_LEN = 3376
TL = 112
NT = 28
NSC = 7          # scatter steps per batch (4 pixel-tiles each)
K4 = 4           # pixel-tiles per local_scatter call
N_GROUP = 464
N_GROUPS = 7
N_CORES = 8
B_LOC = B // N_CORES


def _install_ntff_shim():
    # antenv.axon_hooks is absent in some images; provide it so trace=True
    # (BASS_TRACE=1) can capture NTFF profiles instead of crashing.
    name = "antenv.axon_hooks"
    if name in sys.modules:
        return
    try:
        import antenv  # noqa: F401

        m = types.ModuleType(name)
        m._hook = None
        m.set_axon_ntff_profile_hook = lambda h: setattr(m, "_hook", h)
        m.get_axon_ntff_profile_hook = lambda: m._hook
        sys.modules[name] = m
        setattr(sys.modules["antenv"], "axon_hooks", m)
        from trn_agent_boot.trn_boot import _ntff_profile_via_ctypes

        hook = _ntff_profile_via_ctypes("/opt/axon/libaxon_pjrt.so")
        if hook is not None:
            m.set_axon_ntff_profile_hook(hook)
    except Exception:
        pass


def _build_kernel():
    import concourse.bass as bass
    import concourse.mybir as mybir
    from concourse import bacc, tile
    from concourse.masks import make_identity
    from contextlib import ExitStack

    F32 = mybir.dt.float32
    BF16 = mybir.dt.bfloat16
    I16 = mybir.dt.int16

    nc = bacc.Bacc("TRN2", target_bir_lowering=False, debug=False, num_devices=N_CORES)

    xb = nc.dram_tensor("xb", [B_LOC, HW, C], BF16, kind="ExternalInput")
    wt = nc.dram_tensor("wt", [36, 128, 128], BF16, kind="ExternalInput")
    idxt = nc.dram_tensor("idxt", [128, NSC * K4 * 256], I16, kind="ExternalInput")
    out = nc.dram_tensor("out", [B_LOC, C, HW], F32, kind="ExternalOutput")

    with tile.TileContext(nc) as tc, ExitStack() as ctx:
        const = ctx.enter_context(tc.tile_pool(name="const", bufs=1))
        wsb = const.tile([128, 36 * 128], BF16)
        idxsb = const.tile([128, NSC * 1024], I16)
        ident = const.tile([128, 128], BF16)
        make_identity(nc, ident[:, :])
        # first scatter only needs idx chunk 0 + xin chunk 0: split the
        # const DMAs into chunks and spread them over both HWDGE queues so
        # the pipeline starts ~1.5us in instead of waiting for 4.6MB.
        nc.sync.dma_start(out=idxsb[:, 0:1024], in_=idxt[:, 0:1024])
        nc.scalar.dma_start(
            out=wsb[:, :],
            in_=bass.AP(wt, 0, [[128, 128], [128 * 128, 36], [1, 128]]),
        )

        xin_pool = ctx.enter_context(tc.tile_pool(name="xin", bufs=2))
        xs_pool = ctx.enter_context(tc.tile_pool(name="xs", bufs=2))
        sout_pool = ctx.enter_context(tc.tile_pool(name="sout", bufs=4))
        outst_pool = ctx.enter_context(tc.tile_pool(name="outst", bufs=4))
        tps_pool = ctx.enter_context(tc.tile_pool(name="tps", bufs=2, space="PSUM"))
        mpsum_pool = ctx.enter_context(tc.tile_pool(name="mpsum", bufs=4, space="PSUM"))

        xin_t = [None] * B_LOC

        def issue_xin_dma(b, chunks=1):
            xin_t[b] = xin_pool.tile([128, NT * 256], BF16, name="xin", tag="xin")
            step = NSC // chunks
            for s in range(0, NSC, step):
                nc.sync.dma_start(
                    out=xin_t[b][0:TL, s * 1024 : (s + step) * 1024],
                    in_=bass.AP(
                        xb,
                        b * HW * C + s * K4 * TL * 256,
                        [[256, TL], [TL * 256, step * K4], [1, 256]],
                    ),
                )

        def conv_group(b, xs, oct, g):
            mp = mpsum_pool.tile([128, N_GROUP], F32)
            for i in range(18):
                ct, tap = divmod(i, 9)
                dh, dw = divmod(tap, 3)
                delta = (dh - 1) * PADW + (dw - 1)
                q0 = 59 + g * N_GROUP + delta
                widx = (ct * 9 + tap) * 2 + oct
                nc.tensor.matmul(
                    mp[:, :],
                    lhsT=wsb[:, widx * 128 : (widx + 1) * 128],
                    rhs=xs[:, ct * XS_LEN + q0 : ct * XS_LEN + q0 + N_GROUP],
                    start=(i == 0),
                    stop=(i == 17),
                )
            ost = outst_pool.tile([128, N_GROUP], F32)
            if oct == 0:
                nc.scalar.copy(ost[:, :], mp[:, :])
            else:
                nc.vector.tensor_copy(ost[:, :], mp[:, :])
            nc.sync.dma_start(
                out=out[b, oct * 128 : (oct + 1) * 128, g * 448 : (g + 1) * 448],
                in_=ost[:, :].rearrange("p (r x) -> p r x", r=8)[:, :, 0:56],
            )

        issue_xin_dma(0, chunks=7)
        for s in range(1, NSC):
            nc.sync.dma_start(
                out=idxsb[:, s * 1024 : (s + 1) * 1024],
                in_=idxt[:, s * 1024 : (s + 1) * 1024],
            )
        for b in range(B_LOC):
            if b + 1 < B_LOC:
                issue_xin_dma(b + 1)
            xin = xin_t[b]

            xs = xs_pool.tile([128, 2 * XS_LEN], BF16, name="xs", tag="xs")
            for ct in range(2):
                base = ct * XS_LEN
                nc.vector.memset(xs[:, base : base + PADW], 0.0)
                nc.vector.memset(xs[:, base + 57 * PADW : base + XS_LEN], 0.0)
                nc.vector.memset(
                    xs[:, base + PADW : base + PADW + 56 * PADW].rearrange(
                        "p (r x) -> p r x", r=56
                    )[:, :, 0:1],
                    0.0,
                )
                nc.vector.memset(
                    xs[:, base + PADW + 57 : base + PADW + 57 + 56 * PADW].rearrange(
                        "p (r x) -> p r x", r=56
                    )[:, :, 0:1],
                    0.0,
                )

            for s in range(NSC):
                # conv groups first in program order: the PE streams these
                # 36 matmuls while GPSIMD runs scatter s concurrently.
                if s >= 2:
                    conv_group(b, xs, 0, s - 2)
                    conv_group(b, xs, 1, s - 2)
                sout = sout_pool.tile([128, K4 * 256], BF16, name="sout", tag="sout")
                nc.gpsimd.local_scatter(
                    out_ap=sout[0:TL, :],
                    data_ap=xin[0:TL, s * 1024 : (s + 1) * 1024],
                    idxs_ap=idxsb[0:TL, s * 1024 : (s + 1) * 1024],
                    channels=TL,
                    num_elems=K4 * 256,
                    num_idxs=K4 * 256,
                )
                ps2 = tps_pool.tile([128, 2 * 448], BF16, name="ps2", tag="ps2")
                for ct in range(2):
                    for k in range(K4):
                        nc.tensor.transpose(
                            ps2[:, ct * 448 + k * TL : ct * 448 + (k + 1) * TL],
                            sout[0:TL, k * 256 + ct * 128 : k * 256 + ct * 128 + 128],
                            ident[0:TL, 0:TL],
                        )
                q = 59 + 8 * s * PADW
                for ct in range(2):
                    nc.vector.tensor_copy(
                        xs[:, ct * XS_LEN + q : ct * XS_LEN + q + 8 * PADW].rearrange(
                            "p (r x) -> p r x", r=8
                        )[:, :, 0:56],
                        ps2[:, ct * 448 : (ct + 1) * 448].rearrange(
                            "p (r x) -> p r x", r=8
                        ),
                    )

            for g in (5, 6):
                conv_group(b, xs, 0, g)
                conv_group(b, xs, 1, g)

    nc.compile()
    return nc


def _host_prep(x, w, perm):
    import ml_dtypes

    # [B, C, H, W] -> [B, HW, C] bf16 (pixel-major so scatter tiles DMA
    # straight into [pixel, channel] layout)
    xf = np.ascontiguousarray(
        x.reshape(B, C, HW).transpose(0, 2, 1)
    ).astype(ml_dtypes.bfloat16)

    wtl = np.empty((36, 128, 128), dtype=ml_dtypes.bfloat16)
    wf = np.asarray(w, dtype=np.float32)
    for ct in range(2):
        for tap in range(9):
            kh, kw = divmod(tap, 3)
            for oct in range(2):
                i = (ct * 9 + tap) * 2 + oct
                wtl[i] = wf[
                    oct * 128 : (oct + 1) * 128, ct * 128 : (ct + 1) * 128, kh, kw
                ].T.astype(ml_dtypes.bfloat16)

    iperm = np.empty((HW, C), dtype=np.int16)
    np.put_along_axis(
        iperm, perm.astype(np.int64), np.arange(C, dtype=np.int16)[None, :], axis=1
    )
    idxt = np.zeros((128, NSC * 1024), dtype=np.int16)
    for s in range(NSC):
        for k in range(K4):
            t = K4 * s + k
            idxt[0:TL, s * 1024 + k * 256 : s * 1024 + (k + 1) * 256] = (
                iperm[t * TL : (t + 1) * TL, :] + k * 256
            )

    in_maps = []
    for cidx in range(N_CORES):
        in_maps.append(
            {
                "xb": np.ascontiguousarray(xf[cidx * B_LOC : (cidx + 1) * B_LOC]),
                "wt": wtl,
                "idxt": idxt,
            }
        )
    return in_maps


def kernel(x, w, perm):
    global LAST_RESULT
    _install_ntff_shim()
    from concourse.bass_utils import run_bass_kernel_spmd

    x = np.asarray(x, dtype=np.float32)
    w = np.asarray(w, dtype=np.float32)
    perm = np.asarray(perm)

    if "nc" not in _STATE:
        _STATE["nc"] = _build_kernel()
    nc = _STATE["nc"]

    in_maps = _host_prep(x, w, perm)
    res = run_bass_kernel_spmd(nc, in_maps, core_ids=list(range(N_CORES)))
    LAST_RESULT = res
    out = np.concatenate(
        [r["out"].reshape(B_LOC, C, H, W) for r in res.results], axis=0
    )
    return out.astype(np.float32)
